# revision 17
# baseline (speedup 1.0000x reference)
"""Trainium2 Bass kernel for nn_BottomUpIntegrator (gnn_message_passing).

Sharding: cells split at cluster boundaries across 8 cores (2048 clusters
each); per-core segment sums via one-hot scatter matmuls into PSUM with a
core-invariant static window schedule; cluster+organism phase on-chip; host
combines 12 organism partial floats per core into the final 6 self-model
outputs.

v2: fp8 feats with b1 folded in via a ones row, single ACT table
(tanh-based sigmoid), no in-loop barriers, W=8 scatter windows,
superchunk DMA with 16-32KB descriptors, ACT/Pool/DVE engine split,
vectorized host prep, memoized jit executable.
"""
import os
import numpy as np
import ml_dtypes

import json as _json

from concourse import bass, mybir
from concourse import bass2jax as _b2j
from concourse import bass_utils as _bu
from concourse.tile import TileContext
from concourse.bass_utils import run_bass_kernel_spmd

_orig_compile = _bu.compile_bir_kernel


def _split_waits_compile(bir_json, tmpdir, neff_name="file.neff"):
    """Walrus lowers at most ONE semaphore wait per TPB instruction struct.
    Tile emits several. Hoist extras onto injected same-engine EventSemaphore
    wait instructions immediately before the owner (semantically identical:
    engines execute in program order)."""
    d = _json.loads(bir_json)
    cnt = 0
    for fn in d["functions"]:
        for blk in fn["blocks"]:
            newlist = []
            for ins in blk["instructions"]:
                si = ins.get("sync_info")
                waits = si.get("on_wait", []) if si else []
                if si and len(waits) > 1 and ins.get("opcode") not in (
                        "EventSemaphore",):
                    for w_i, w in enumerate(waits[:-1]):
                        cnt += 1
                        newlist.append({
                            "debug": ins.get("debug", 0),
                            "engine": ins["engine"],
                            "ins": [], "outs": [],
                            "name": f"{ins['name']}-wsplit{w_i}",
                            "opcode": "EventSemaphore",
                            "sync_info": {"on_update": [], "on_wait": [w]},
                        })
                    si["on_wait"] = [waits[-1]]
                newlist.append(ins)
            blk["instructions"] = newlist
    print(f"[wait-split] hoisted {cnt} extra waits")
    return _orig_compile(_json.dumps(d).encode(), tmpdir, neff_name=neff_name)


_bu.compile_bir_kernel = _split_waits_compile
_b2j.compile_bir_kernel = _split_waits_compile

F32 = mybir.dt.float32
BF16 = mybir.dt.bfloat16
FP8 = mybir.dt.float8e4
AF = mybir.ActivationFunctionType
OP = mybir.AluOpType
AX = mybir.AxisListType

NCORES = 8
KLOC = 2048            # clusters per core
NPAD = 262144          # padded cells per core
CHUNK = 8192           # cells per chunk
NCHUNK = NPAD // CHUNK # 32
SUPER = 4              # chunks per DMA superchunk
NSUP = NCHUNK // SUPER # 8
NTILES = NPAD // 128   # 2048 scatter tiles per core
TPB = NTILES // 4      # tiles per 512-cluster block
PADSEG = 1.0e9

FP8_NP = ml_dtypes.float8_e4m3
BF16_NP = ml_dtypes.bfloat16


def _window_starts(W):
    S = np.arange(NTILES)
    s = S % TPB
    return np.clip(s - W // 2, 0, TPB - W).astype(np.int64)


def build_program(W):
    nc = bass.Bass(trn_type="TRN2", use_seq_codegen=True)
    featsT = nc.dram_tensor("featsT", [80, NPAD // 2], FP8, kind="ExternalInput")
    archcm = nc.dram_tensor("archcm", [128, NCHUNK * 512], BF16, kind="ExternalInput")
    cellvec = nc.dram_tensor("cellvec", [128, NCHUNK * 256], BF16, kind="ExternalInput")
    w1d = nc.dram_tensor("w1d", [80, 128], FP8, kind="ExternalInput")
    w2d = nc.dram_tensor("w2d", [128, 2], BF16, kind="ExternalInput")
    b2hd = nc.dram_tensor("b2hd", [128, 1], F32, kind="ExternalInput")
    iotat = nc.dram_tensor("iotat", [128, 64 * W], BF16, kind="ExternalInput")
    ident = nc.dram_tensor("ident", [128, 128], F32, kind="ExternalInput")
    v1 = nc.dram_tensor("v1", [7, 32], F32, kind="ExternalInput")
    c1b = nc.dram_tensor("c1b", [32, 1], F32, kind="ExternalInput")
    v2 = nc.dram_tensor("v2", [32, 1], F32, kind="ExternalInput")
    c2h = nc.dram_tensor("c2h", [1, 1], F32, kind="ExternalInput")
    out_cluster = nc.dram_tensor("out_cluster", [KLOC, 8], F32, kind="ExternalOutput")
    out_org = nc.dram_tensor("out_org", [1, 12], F32, kind="ExternalOutput")

    wstart = _window_starts(W)

    with TileContext(nc) as tc:
        with (
            tc.tile_pool(name="const", bufs=1) as cp,
            tc.tile_pool(name="feats", bufs=4) as fp,
            tc.tile_pool(name="acp", bufs=4) as ap_,
            tc.tile_pool(name="cvp", bufs=4) as vp_,
            tc.tile_pool(name="hs", bufs=2) as hp,
            tc.tile_pool(name="small", bufs=4) as sp,
            tc.tile_pool(name="scatv", bufs=2) as vp,
            tc.tile_pool(name="ohp", bufs=2) as op_,
            tc.tile_pool(name="ph_b", bufs=1) as bp,
            tc.tile_pool(name="scatps", bufs=1, space="PSUM") as pps,
        ):
            # ---- constants ----------------------------------------------
            w1s = cp.tile([80, 128], FP8, tag="w1s")
            nc.sync.dma_start(out=w1s[:], in_=w1d[:])
            w2s = cp.tile([128, 2], BF16, tag="w2s")
            nc.sync.dma_start(out=w2s[:], in_=w2d[:])
            b2hs = cp.tile([128, 1], F32, tag="b2hs")
            nc.sync.dma_start(out=b2hs[:], in_=b2hd[:])
            iots = cp.tile([128, 64 * W], BF16, tag="iots")
            nc.sync.dma_start(out=iots[:], in_=iotat[:])
            ids = cp.tile([128, 128], F32, tag="ids")
            nc.sync.dma_start(out=ids[:], in_=ident[:])
            v1s = cp.tile([7, 32], F32, tag="v1s")
            nc.sync.dma_start(out=v1s[:], in_=v1[:])
            c1s = cp.tile([32, 1], F32, tag="c1s")
            nc.sync.dma_start(out=c1s[:], in_=c1b[:])
            v2s = cp.tile([32, 1], F32, tag="v2s")
            nc.sync.dma_start(out=v2s[:], in_=v2[:])
            c2hs = cp.tile([1, 1], F32, tag="c2hs")
            nc.sync.dma_start(out=c2hs[:], in_=c2h[:])
            ones = cp.tile([128, 1], F32, tag="ones")
            nc.vector.memset(ones[:], 1.0)

            zbf = cp.tile([128, 512], BF16, tag="zbf")
            nc.vector.memset(zbf[:], 0.0)

            # Pre-touch DMA-loaded constants on their consuming engines.
            scra = cp.tile([128, 4], F32, tag="scra")
            nc.scalar.activation(out=scra[:, 0:1], in_=b2hs[:], func=AF.Copy)
            nc.scalar.activation(out=scra[0:32, 1:2], in_=c1s[:], func=AF.Copy)
            nc.scalar.activation(out=scra[0:1, 2:3], in_=c2hs[:], func=AF.Copy)
            scrv = cp.tile([1, 2], F32, tag="scrv")
            nc.vector.tensor_copy(out=scrv[:, 0:1], in_=iots[0:1, 0:1])
            nc.gpsimd.tensor_copy(out=scrv[:, 1:2], in_=iots[0:1, 1:2])

            # persistent scatter accumulators: 4 PSUM banks of [15, 512]
            scat = [pps.tile([15, 512], F32, tag=f"scat{b}", name=f"scat{b}")
                    for b in range(4)]
            nc.tensor.matmul(out=scat[0][0:1, 0:1], lhsT=ids[0:1, 0:1],
                             rhs=ids[0:1, 0:1], start=True, stop=True,
                             skip_group_check=True)
            for b in range(4):
                nc.tensor.matmul(out=scat[b][:], lhsT=zbf[:, 0:15], rhs=zbf[:],
                                 start=True, stop=False, skip_group_check=True)

            # ---- phase A ------------------------------------------------
            with (
                tc.tile_pool(name="mm1ps", bufs=2, space="PSUM") as pp1,
                tc.tile_pool(name="mm2ps", bufs=2, space="PSUM") as pp2,
            ):
                for s in range(NSUP):
                    ft = fp.tile([80, SUPER * 4096], FP8, tag="ft")
                    c0 = s * SUPER * 4096
                    if s == 0:
                        # chunk-granular first loads so compute starts sooner
                        for i4 in range(SUPER):
                            nc.sync.dma_start(
                                out=ft[:, i4 * 4096:(i4 + 1) * 4096],
                                in_=featsT[:, c0 + i4 * 4096:c0 + (i4 + 1) * 4096],
                                max_dma_last_dim=4096)
                    else:
                        # <=8KB descriptors: 16KB packets pin to one DMA engine
                        nc.sync.dma_start(
                            out=ft[:], in_=featsT[:, c0:c0 + SUPER * 4096],
                            max_dma_last_dim=8192)
                    ac = ap_.tile([128, SUPER * 512], BF16, tag="ac")
                    nc.sync.dma_start(
                        out=ac[:], in_=archcm[:, s * SUPER * 512:(s + 1) * SUPER * 512])
                    cv = vp_.tile([128, SUPER * 256], BF16, tag="cv")
                    nc.sync.dma_start(
                        out=cv[:], in_=cellvec[:, s * SUPER * 256:(s + 1) * SUPER * 256])
                    # pad DMA count so the 8-lane round-robin stays aligned
                    # with bufs=4 buffer reuse (4 per super; 8 for super 0)
                    dmy = sp.tile([1, 1], F32, tag="dmy")
                    nc.sync.dma_start(out=dmy[:], in_=c2h[:])
                    if s == 0:
                        dmy2 = sp.tile([1, 1], F32, tag="dmy2")
                        nc.sync.dma_start(out=dmy2[:], in_=c2h[:])

                    for i in range(SUPER):
                        k = s * SUPER + i
                        fcol = i * 4096
                        acs = ac[:, i * 512:(i + 1) * 512]
                        cvs = cv[:, i * 256:(i + 1) * 256]

                        # mm1 + relu -> hs [128, 4096] bf16 (b1 folded in)
                        hs = hp.tile([128, 4096], BF16, tag="hs")
                        for j in range(8):
                            hp1 = pp1.tile([128, 512], F32, tag="hp1")
                            nc.tensor.matmul(out=hp1[:], lhsT=w1s[:],
                                             rhs=ft[:, fcol + 512 * j:fcol + 512 * (j + 1)],
                                             start=True, stop=True)
                            dst = hs[:, 512 * j:512 * (j + 1)]
                            if j < 5:
                                nc.scalar.activation(out=dst, in_=hp1[:], func=AF.Relu)
                            else:
                                nc.vector.tensor_scalar(out=dst, in0=hp1[:],
                                                        scalar1=0.0, scalar2=None,
                                                        op0=OP.max)

                        # mm2 -> base preact [128, 64] cell-major in PSUM
                        bb = pp2.tile([128, 64], F32, tag="bb")
                        for t in range(32):
                            nc.tensor.matmul(out=bb[:, 2 * t:2 * t + 2],
                                             lhsT=hs[:, 128 * t:128 * (t + 1)], rhs=w2s[:],
                                             start=(t == 0), stop=(t == 31),
                                             skip_group_check=True)

                        # sigmoid via tanh: sig = 0.5*tanh(0.5 x + 0.5 b2) + 0.5
                        sg = sp.tile([128, 64], F32, tag="sg")
                        nc.scalar.activation(out=sg[:], in_=bb[:], func=AF.Tanh,
                                             bias=b2hs[:], scale=0.5)
                        # imp = clip((t+1) * eph2, .01, 1);  w = imp * eph
                        imp = sp.tile([128, 64], F32, tag="imp")
                        nc.vector.scalar_tensor_tensor(
                            out=imp[:], in0=sg[:], scalar=1.0,
                            in1=cvs[:, 64:128], op0=OP.add, op1=OP.mult)
                        nc.vector.tensor_scalar(out=imp[:], in0=imp[:], scalar1=0.01,
                                                scalar2=1.0, op0=OP.max, op1=OP.min)
                        wq = sp.tile([128, 64], F32, tag="wq")
                        nc.vector.tensor_tensor(out=wq[:], in0=imp[:], in1=cvs[:, 0:64],
                                                op=OP.mult)
                        ew = sp.tile([128, 64], F32, tag="ew")
                        nc.scalar.activation(out=ew[:], in_=wq[:], func=AF.Exp)

                        # values [128, 64*15] bf16
                        # value-major vt: [128, 15v, 64s] -> contiguous writes
                        vt = vp.tile([128, 15 * 64], BF16, tag="vt")
                        vv = vt[:].rearrange("p (v s) -> p v s", s=64)
                        nc.gpsimd.memset(vv[:, 0:1, :], 1.0)
                        nc.gpsimd.tensor_copy(
                            out=vv[:, 1:2, :],
                            in_=ew[:].to_broadcast([128, 64, 1]).rearrange(
                                "p s o -> p o s"))
                        nc.vector.tensor_tensor(
                            out=vv[:, 2:6, :],
                            in0=acs[:, 0:256].rearrange("p (a s) -> p a s", s=64),
                            in1=ew[:].to_broadcast([128, 64, 4]).rearrange(
                                "p s a -> p a s"),
                            op=OP.mult)
                        nc.gpsimd.tensor_copy(out=vt[:, 6 * 64:10 * 64],
                                              in_=acs[:, 0:256])
                        nc.gpsimd.tensor_copy(out=vt[:, 10 * 64:14 * 64],
                                              in_=acs[:, 256:512])
                        nc.gpsimd.tensor_copy(out=vt[:, 14 * 64:15 * 64],
                                              in_=cvs[:, 128:192])

                        # one-hot [128, 64*W] bf16 (DVE)
                        oh = op_.tile([128, 64 * W], BF16, tag="oh")
                        ohv = oh[:].rearrange("p (s w) -> p s w", w=W)
                        iov = iots[:].rearrange("p (s w) -> p s w", w=W)
                        nc.vector.tensor_tensor(
                            out=ohv[:, :, :], in0=iov[:, :, :],
                            in1=cvs[:, 192:256].to_broadcast([128, 64, W]),
                            op=OP.is_equal)

                        # scatter: col j -> sorted tile S = 64k + 32*(j%2) + j//2
                        # lhsT strided: value-major vt, tile j's 15 values at
                        # stride 64
                        for j in range(64):
                            S = 64 * k + 32 * (j % 2) + (j // 2)
                            blk = S // TPB
                            f = int(wstart[S])
                            nc.tensor.matmul(out=scat[blk][:, f:f + W],
                                             lhsT=vv[:, :, j:j + 1],
                                             rhs=oh[:, W * j:W * j + W],
                                             start=False,
                                             stop=(j == 63 and k % 8 == 7),
                                             skip_group_check=True)

            # ---- phase B ------------------------------------------------
            tc.strict_bb_all_engine_barrier()
            sc = bp.tile([15, 2048], F32, tag="sc")
            for b in range(4):
                nc.vector.tensor_copy(out=sc[:, 512 * b:512 * (b + 1)], in_=scat[b][:])

            with (
                tc.tile_pool(name="ptps", bufs=2, space="PSUM") as ppt,
                tc.tile_pool(name="mmbps", bufs=2, space="PSUM") as ppm,
            ):
                tt = bp.tile([128, 16 * 15], F32, tag="tt")
                for b in range(16):
                    pt = ppt.tile([128, 15], F32, tag="pt")
                    nc.tensor.transpose(out=pt[:], in_=sc[:, 128 * b:128 * (b + 1)],
                                        identity=ids[0:15, 0:15])
                    nc.vector.tensor_copy(out=tt[:, 15 * b:15 * (b + 1)], in_=pt[:])
                tv = tt[:].rearrange("p (b q) -> p b q", q=15)
                cnt = tv[:, :, 0:1]      # [128,16,1]
                sew = tv[:, :, 1:2]
                sewa = tv[:, :, 2:6]
                sa = tv[:, :, 6:10]
                ssq = tv[:, :, 10:14]
                ssur = tv[:, :, 14:15]

                def wt(tag):
                    return bp.tile([128, 16], F32, tag=tag, name=tag)

                def v3(t):
                    return t[:].rearrange("p (b a) -> p b a", a=1)

                def w4(tag):
                    t = bp.tile([128, 64], F32, tag=tag, name=tag)
                    return t, t[:].rearrange("p (b a) -> p b a", a=4)

                cntc = wt("cntc")
                nc.vector.tensor_scalar(out=v3(cntc), in0=cnt, scalar1=1.0,
                                        scalar2=None, op0=OP.max)
                rc = wt("rc")
                nc.vector.reciprocal(out=rc[:], in_=cntc[:])
                den = wt("den")
                nc.vector.tensor_scalar(out=v3(den), in0=sew, scalar1=1.0,
                                        scalar2=None, op0=OP.max)
                rden = wt("rden")
                nc.vector.reciprocal(out=rden[:], in_=den[:])
                agr, agrv = w4("agr")
                nc.vector.tensor_tensor(out=agrv, in0=sewa,
                                        in1=rden[:].to_broadcast([128, 16, 4]),
                                        op=OP.mult)
                mx = wt("mx")
                nc.vector.tensor_reduce(out=v3(mx), in_=agrv, axis=AX.X, op=OP.max)
                es, esv = w4("es")
                nc.vector.tensor_tensor(out=esv, in0=agrv,
                                        in1=mx[:].to_broadcast([128, 16, 4]),
                                        op=OP.subtract)
                nc.scalar.activation(out=es[:], in_=es[:], func=AF.Exp)
                ssum = wt("ssum")
                nc.vector.tensor_reduce(out=v3(ssum), in_=esv, axis=AX.X, op=OP.add)
                rssum = wt("rssum")
                nc.vector.reciprocal(out=rssum[:], in_=ssum[:])
                agg, aggv = w4("agg")
                nc.vector.tensor_tensor(out=aggv, in0=esv,
                                        in1=rssum[:].to_broadcast([128, 16, 4]),
                                        op=OP.mult)
                mean, meanv = w4("mean")
                nc.vector.tensor_tensor(out=meanv, in0=sa,
                                        in1=rc[:].to_broadcast([128, 16, 4]),
                                        op=OP.mult)
                var, varv = w4("var")
                nc.vector.tensor_tensor(out=varv, in0=meanv, in1=meanv, op=OP.mult)
                cntb = wt("cntb")
                nc.vector.tensor_copy(out=v3(cntb), in_=cnt)
                nc.vector.tensor_tensor(out=varv, in0=varv,
                                        in1=cntb[:].to_broadcast([128, 16, 4]),
                                        op=OP.mult)
                nc.vector.tensor_tensor(out=varv, in0=ssq, in1=varv, op=OP.subtract)
                cm1 = wt("cm1")
                nc.vector.tensor_scalar(out=v3(cm1), in0=cnt, scalar1=-1.0,
                                        scalar2=1.0, op0=OP.add, op1=OP.max)
                rcm1 = wt("rcm1")
                nc.vector.reciprocal(out=rcm1[:], in_=cm1[:])
                nc.vector.tensor_tensor(out=varv, in0=varv,
                                        in1=rcm1[:].to_broadcast([128, 16, 4]),
                                        op=OP.mult)
                vm = wt("vm")
                nc.vector.tensor_reduce(out=v3(vm), in_=varv, axis=AX.X, op=OP.add)
                nc.vector.tensor_scalar(out=vm[:], in0=vm[:], scalar1=0.25,
                                        scalar2=None, op0=OP.mult)
                phic = wt("phic")
                nc.vector.tensor_scalar(out=phic[:], in0=vm[:], scalar1=2.0,
                                        scalar2=1.0, op0=OP.mult, op1=OP.min)
                nc.vector.tensor_scalar(out=phic[:], in0=phic[:], scalar1=-1.0,
                                        scalar2=1.0, op0=OP.mult, op1=OP.add)
                coh = wt("coh")
                nc.vector.tensor_scalar(out=coh[:], in0=vm[:], scalar1=-1.0,
                                        scalar2=1.0, op0=OP.mult, op1=OP.add)
                perr = wt("perr")
                nc.vector.tensor_tensor(out=v3(perr), in0=ssur, in1=v3(rc),
                                        op=OP.mult)
                integ = wt("integ")
                nc.vector.tensor_scalar(out=integ[:], in0=perr[:], scalar1=-1.0,
                                        scalar2=1.0, op0=OP.mult, op1=OP.add)
                nc.vector.tensor_tensor(out=integ[:], in0=integ[:], in1=phic[:],
                                        op=OP.mult)

                # cluster MLP
                cft = bp.tile([128, 16 * 7], F32, tag="cft")
                cfv = cft[:].rearrange("p (b q) -> p b q", q=7)
                nc.vector.tensor_copy(out=cfv[:, :, 0:4], in_=aggv)
                nc.vector.tensor_copy(out=cfv[:, :, 4:5],
                                      in_=phic[:].to_broadcast([128, 16, 1]))
                nc.vector.tensor_copy(out=cfv[:, :, 5:6],
                                      in_=coh[:].to_broadcast([128, 16, 1]))
                szf = wt("szf")
                nc.vector.tensor_scalar(out=v3(szf), in0=cnt, scalar1=0.05,
                                        scalar2=1.0, op0=OP.mult, op1=OP.min)
                nc.vector.tensor_copy(out=cfv[:, :, 6:7],
                                      in_=szf[:].to_broadcast([128, 16, 1]))
                cftt = bp.tile([7, 2048], F32, tag="cftt")
                for b in range(16):
                    ptc = ppt.tile([128, 128], F32, tag="pt")
                    nc.tensor.transpose(out=ptc[0:7, :],
                                        in_=cft[:, 7 * b:7 * (b + 1)], identity=ids[:])
                    nc.vector.tensor_copy(out=cftt[:, 128 * b:128 * (b + 1)],
                                          in_=ptc[0:7, :])
                hcs = bp.tile([32, 2048], F32, tag="hcs")
                for i in range(4):
                    hcp = ppm.tile([32, 512], F32, tag="mmp")
                    nc.tensor.matmul(out=hcp[:], lhsT=v1s[:],
                                     rhs=cftt[:, 512 * i:512 * (i + 1)],
                                     start=True, stop=True)
                    nc.scalar.activation(out=hcs[:, 512 * i:512 * (i + 1)],
                                         in_=hcp[:], func=AF.Relu, bias=c1s[:])
                # basec tanh preact
                sgc = bp.tile([1, 2048], F32, tag="sgc")
                for i in range(4):
                    bcp = ppm.tile([32, 512], F32, tag="mmp")
                    nc.tensor.matmul(out=bcp[0:1, :], lhsT=v2s[:],
                                     rhs=hcs[:, 512 * i:512 * (i + 1)],
                                     start=True, stop=True)
                    nc.scalar.activation(out=sgc[:, 512 * i:512 * (i + 1)],
                                         in_=bcp[0:1, :], func=AF.Tanh,
                                         bias=c2hs[:], scale=0.5)
                tct = wt("tct")
                for b in range(16):
                    ptb = ppt.tile([128, 128], F32, tag="pt")
                    nc.tensor.transpose(out=ptb[:, 0:1],
                                        in_=sgc[:, 128 * b:128 * (b + 1)],
                                        identity=ids[0:1, 0:1])
                    nc.vector.tensor_copy(out=tct[:, b:b + 1], in_=ptb[:, 0:1])
                # impc = clip(0.5*(t+1)*phic, .01, 1)
                impc = wt("impc")
                nc.vector.scalar_tensor_tensor(out=impc[:], in0=tct[:], scalar=1.0,
                                               in1=phic[:], op0=OP.add, op1=OP.mult)
                nc.vector.tensor_scalar(out=impc[:], in0=impc[:], scalar1=0.5,
                                        scalar2=0.01, op0=OP.mult, op1=OP.max)
                nc.vector.tensor_scalar(out=impc[:], in0=impc[:], scalar1=1.0,
                                        scalar2=None, op0=OP.min)
                valid = wt("valid")
                nc.vector.tensor_scalar(out=v3(valid), in0=cnt, scalar1=0.0,
                                        scalar2=None, op0=OP.is_gt)
                eimp = wt("eimp")
                nc.scalar.activation(out=eimp[:], in_=impc[:], func=AF.Exp)
                nc.vector.tensor_tensor(out=eimp[:], in0=eimp[:], in1=valid[:],
                                        op=OP.mult)
                amx = wt("amx")
                nc.vector.tensor_reduce(out=v3(amx), in_=aggv, axis=AX.X, op=OP.max)
                bsel, bselv = w4("bsel")
                nc.vector.tensor_tensor(out=bselv, in0=aggv,
                                        in1=amx[:].to_broadcast([128, 16, 4]),
                                        op=OP.is_equal)
                taken = wt("taken")
                nc.vector.memset(taken[:], 0.0)
                notk = wt("notk")
                for a in range(4):
                    nc.vector.tensor_scalar(out=notk[:], in0=taken[:], scalar1=-1.0,
                                            scalar2=1.0, op0=OP.mult, op1=OP.add)
                    nc.vector.tensor_tensor(out=bselv[:, :, a:a + 1],
                                            in0=bselv[:, :, a:a + 1], in1=v3(notk),
                                            op=OP.mult)
                    if a < 3:
                        nc.vector.tensor_tensor(out=v3(taken), in0=v3(taken),
                                                in1=bselv[:, :, a:a + 1], op=OP.max)
                # reductions -> R [128, 12]
                r = bp.tile([128, 12], F32, tag="r")
                ga, gav = w4("ga")
                nc.vector.tensor_tensor(out=gav, in0=aggv,
                                        in1=eimp[:].to_broadcast([128, 16, 4]),
                                        op=OP.mult)
                pv = wt("pv")
                nc.vector.tensor_tensor(out=pv[:], in0=phic[:], in1=valid[:],
                                        op=OP.mult)
                cvv = wt("cvv")
                nc.vector.tensor_tensor(out=cvv[:], in0=coh[:], in1=valid[:],
                                        op=OP.mult)
                bv, bvv = w4("bv")
                nc.vector.tensor_tensor(out=bvv, in0=bselv,
                                        in1=valid[:].to_broadcast([128, 16, 4]),
                                        op=OP.mult)
                nc.vector.tensor_reduce(out=r[:, 0:1], in_=eimp[:], axis=AX.X,
                                        op=OP.add)
                gat = ga[:].rearrange("p (b a) -> p a b", a=4)
                nc.vector.tensor_reduce(
                    out=r[:, 1:5].rearrange("p (a o) -> p a o", o=1),
                    in_=gat, axis=AX.X, op=OP.add)
                nc.vector.tensor_reduce(out=r[:, 5:6], in_=pv[:], axis=AX.X, op=OP.add)
                nc.vector.tensor_reduce(out=r[:, 6:7], in_=cvv[:], axis=AX.X,
                                        op=OP.add)
                nc.vector.tensor_reduce(out=r[:, 7:8], in_=valid[:], axis=AX.X,
                                        op=OP.add)
                bvt = bv[:].rearrange("p (b a) -> p a b", a=4)
                nc.vector.tensor_reduce(
                    out=r[:, 8:12].rearrange("p (a o) -> p a o", o=1),
                    in_=bvt, axis=AX.X, op=OP.add)
                orgp = ppm.tile([32, 512], F32, tag="mmp")
                nc.tensor.matmul(out=orgp[0:1, 0:12], lhsT=ones[:], rhs=r[:],
                                 start=True, stop=True)
                orgs = bp.tile([1, 12], F32, tag="orgs")
                nc.vector.tensor_copy(out=orgs[:], in_=orgp[0:1, 0:12])
                nc.sync.dma_start(out=out_org[:], in_=orgs[:])

                # cluster_out [2048, 8]
                oc = bp.tile([128, 128], F32, tag="oc")
                ocv = oc[:].rearrange("p (b q) -> p b q", q=8)
                nc.vector.tensor_copy(out=ocv[:, :, 0:4], in_=aggv)
                nc.vector.tensor_copy(out=ocv[:, :, 4:5],
                                      in_=phic[:].to_broadcast([128, 16, 1]))
                nc.vector.tensor_copy(out=ocv[:, :, 5:6],
                                      in_=coh[:].to_broadcast([128, 16, 1]))
                nc.vector.tensor_copy(out=ocv[:, :, 6:7],
                                      in_=perr[:].to_broadcast([128, 16, 1]))
                nc.vector.tensor_copy(out=ocv[:, :, 7:8],
                                      in_=integ[:].to_broadcast([128, 16, 1]))
                nc.sync.dma_start(
                    out=out_cluster[:].rearrange("(b p) q -> p b q", p=128), in_=ocv)
    return nc


_NC_CACHE = {}


def _get_program(W):
    if W not in _NC_CACHE:
        _NC_CACHE[W] = build_program(W)
    return _NC_CACHE[W]


# --------------------------------------------------------------------------
# host-side prep (vectorized)
# --------------------------------------------------------------------------

def _schedule_core(lseg, W):
    """Static window schedule for one core. Returns (dst, rel, ok)."""
    Nc = lseg.shape[0]
    S = np.arange(NTILES)
    f = _window_starts(W)
    wlo = 512 * (S // TPB) + f
    whi = wlo + W
    a = np.searchsorted(lseg, whi)
    b = a - 128 * S
    runmin = np.minimum.accumulate(b)
    cur_next = np.minimum(runmin + 128 * S, 128 * (S + 1))
    cur = np.concatenate([[0], cur_next[:-1]])
    take = (cur_next - cur).astype(np.int64)
    if cur_next[-1] != Nc:
        return None, None, False
    m = take > 0
    first_idx = np.minimum(cur, Nc - 1)
    if np.any((lseg[first_idx] < wlo) & m):
        return None, None, False
    tile_of = np.repeat(S, take)
    dst = np.repeat(128 * S - cur, take) + np.arange(Nc)
    rel = (lseg - wlo[tile_of]).astype(np.float32)
    return dst, rel, True


def _prep_inputs(state, arch, energy, phi_local, surprise, seg_ids, W,
                 W1, b1, W2, b2, V1, c1, V2, c2):
    N = state.shape[0]
    fp8_one = np.float32(1.0).astype(FP8_NP).view(np.uint8)

    # global conversions (once)
    f8s = state.astype(FP8_NP).view(np.uint8)          # [N, 32]
    f8a = arch.astype(FP8_NP).view(np.uint8)           # [N, 4]
    arch_bf = arch.astype(BF16_NP).view(np.uint16)     # [N, 4]
    archsq_bf = (arch * arch).astype(BF16_NP).view(np.uint16)
    eph = energy * phi_local
    eph_bf = eph.astype(BF16_NP).view(np.uint16)
    eph2_bf = (0.5 * eph).astype(BF16_NP).view(np.uint16)
    su_bf = surprise.astype(BF16_NP).view(np.uint16)

    w1d = np.zeros((80, 128), np.float32)
    w1d[0:36, 0:64] = W1
    w1d[36:72, 64:128] = W1
    w1d[72, 0:64] = b1
    w1d[72, 64:128] = b1
    w2d = np.zeros((128, 2), np.float32)
    w2d[0:64, 0] = W2[:, 0]
    w2d[64:128, 1] = W2[:, 0]
    iota = np.tile(np.arange(W, dtype=np.float32), 64)
    consts = dict(
        w1d=w1d.astype(FP8_NP),
        w2d=w2d.astype(BF16_NP),
        b2hd=np.full((128, 1), 0.5 * b2[0], np.float32),
        iotat=np.ascontiguousarray(
            np.broadcast_to(iota.astype(BF16_NP), (128, 64 * W))),
        ident=np.eye(128, dtype=np.float32),
        v1=V1, c1b=c1.reshape(32, 1), v2=V2,
        c2h=np.full((1, 1), 0.5 * c2[0], np.float32),
    )

    bounds = np.searchsorted(seg_ids, np.arange(0, 16384 + 1, KLOC))
    in_maps = []
    for c in range(NCORES):
        B0, B1 = int(bounds[c]), int(bounds[c + 1])
        lseg = (seg_ids[B0:B1] - KLOC * c).astype(np.int64)
        dst, rel, ok = _schedule_core(lseg, W)
        if not ok:
            return None, None
        Nc = B1 - B0

        # padded feats [NPAD, 36] fp8-bytes -> featsT [73, NPAD//2]
        pad36 = np.zeros((NPAD, 36), np.uint8)
        pad36[dst, :32] = f8s[B0:B1]
        pad36[dst, 32:] = f8a[B0:B1]
        fT = np.zeros((80, NPAD // 2), np.uint8)
        src = pad36.reshape(NCHUNK, 2, 4096, 36).transpose(1, 3, 0, 2)
        fTv = fT[:72].reshape(2, 36, NCHUNK, 4096)
        np.copyto(fTv, src)
        fT[72] = fp8_one

        # archcm [128, NCHUNK*512] bf16: per chunk, value-major
        # [a0..a3 rows of 64 | asq0..asq3 rows of 64]
        acm_u = np.empty((128, NCHUNK, 2, 4, 64), np.uint16)
        apad = np.zeros((NPAD, 4), np.uint16)
        apad[dst] = arch_bf[B0:B1]
        acm_u[:, :, 0] = (apad.reshape(NCHUNK, 2, 32, 128, 4)
                          .transpose(3, 0, 4, 2, 1).reshape(128, NCHUNK, 4, 64))
        apad[:] = 0
        apad[dst] = archsq_bf[B0:B1]
        acm_u[:, :, 1] = (apad.reshape(NCHUNK, 2, 32, 128, 4)
                          .transpose(3, 0, 4, 2, 1).reshape(128, NCHUNK, 4, 64))
        acm = acm_u.reshape(128, NCHUNK * 512).view(BF16_NP)

        # cellvec [128, NCHUNK*256] bf16: [eph | eph2 | su | rel] per chunk
        def swz(x_pad):
            return np.ascontiguousarray(
                x_pad.reshape(NCHUNK, 2, 32, 128).transpose(3, 0, 2, 1)
            ).reshape(128, NCHUNK, 64)

        cvb = np.empty((128, NCHUNK, 4, 64), np.uint16)
        epad = np.zeros(NPAD, np.uint16)
        epad[dst] = eph_bf[B0:B1]
        cvb[:, :, 0, :] = swz(epad)
        epad2 = np.zeros(NPAD, np.uint16)
        epad2[dst] = eph2_bf[B0:B1]
        cvb[:, :, 1, :] = swz(epad2)
        spad = np.zeros(NPAD, np.uint16)
        spad[dst] = su_bf[B0:B1]
        cvb[:, :, 2, :] = swz(spad)
        rpad = np.full(NPAD, PADSEG, np.float32)
        rpad[dst] = rel
        cvb[:, :, 3, :] = swz(rpad.astype(BF16_NP).view(np.uint16))
        cvv = cvb.reshape(128, NCHUNK * 256).view(BF16_NP)

        in_maps.append(dict(featsT=fT.view(FP8_NP),
                            archcm=acm,
                            cellvec=np.ascontiguousarray(cvv), **consts))
    return in_maps, True


# --------------------------------------------------------------------------
# cached PJRT execution (avoids re-trace/re-compile on repeat calls)
# --------------------------------------------------------------------------

_EXEC_CACHE = {}


def _build_exec(nc):
    import jax
    from concourse.bass2jax import (_bass_exec_p, install_neuronx_cc_hook,
                                    partition_id_tensor)
    from jax.sharding import Mesh, PartitionSpec
    try:
        from jax.experimental.shard_map import shard_map
    except ImportError:
        from jax import shard_map

    install_neuronx_cc_hook()
    partition_name = nc.partition_id_tensor.name if nc.partition_id_tensor else None
    in_names = []
    out_names = []
    out_avals = []
    zero_shapes = []
    for alloc in nc.m.functions[0].allocations:
        if not isinstance(alloc, mybir.MemoryLocationSet):
            continue
        name = alloc.memorylocations[0].name
        if alloc.kind == "ExternalInput":
            if name != partition_name:
                in_names.append(name)
        elif alloc.kind == "ExternalOutput":
            out_names.append(name)
            shape = tuple(alloc.tensor_shape)
            dtype = mybir.dt.np(alloc.dtype)
            out_avals.append(jax.core.ShapedArray(shape, dtype))
            zero_shapes.append((shape, dtype))
    n_params = len(in_names)
    n_outs = len(out_names)
    all_names = in_names + out_names
    if partition_name is not None:
        all_names = all_names + [partition_name]

    def _body(*args):
        operands = list(args)
        if partition_name is not None:
            operands.append(partition_id_tensor())
        outs = _bass_exec_p.bind(
            *operands,
            out_avals=tuple(out_avals),
            in_names=tuple(all_names),
            out_names=tuple(out_names),
            lowering_input_output_aliases=(),
            sim_require_finite=True,
            sim_require_nnan=True,
            nc=nc,
        )
        return tuple(outs)

    devices = jax.devices()[:NCORES]
    mesh = Mesh(np.asarray(devices), ("core",))
    in_specs = (PartitionSpec("core"),) * (n_params + n_outs)
    out_specs = (PartitionSpec("core"),) * n_outs
    donate = tuple(range(n_params, n_params + n_outs))
    sharded = jax.jit(
        shard_map(_body, mesh=mesh, in_specs=in_specs, out_specs=out_specs,
                  check_rep=False),
        donate_argnums=donate, keep_unused=True)

    def run(in_maps):
        concat_in = [
            np.concatenate([np.asarray(in_maps[c][name]) for c in range(NCORES)],
                           axis=0)
            for name in in_names
        ]
        concat_zeros = [
            np.zeros((NCORES * s[0], *s[1:]), d) for s, d in zero_shapes
        ]
        out_arrs = sharded(*concat_in, *concat_zeros)
        return [
            {name: np.asarray(out_arrs[i]).reshape(NCORES, *out_avals[i].shape)[c]
             for i, name in enumerate(out_names)}
            for c in range(NCORES)
        ]

    return run


def _run(nc, in_maps, W):
    if os.environ.get("BASS_TRACE_RUN"):
        res = run_bass_kernel_spmd(nc, in_maps, list(range(NCORES)))
        return res.results
    if W not in _EXEC_CACHE:
        _EXEC_CACHE[W] = _build_exec(nc)
    return _EXEC_CACHE[W](in_maps)


def kernel(state, arch, energy, phi_local, surprise, seg_ids, n_clusters,
           W1, b1, W2, b2, V1, c1, V2, c2):
    state = np.asarray(state, np.float32)
    arch = np.asarray(arch, np.float32)
    energy = np.asarray(energy, np.float32)
    phi_local = np.asarray(phi_local, np.float32)
    surprise = np.asarray(surprise, np.float32)
    seg_ids = np.asarray(seg_ids)
    W1 = np.asarray(W1, np.float32); b1 = np.asarray(b1, np.float32)
    W2 = np.asarray(W2, np.float32); b2 = np.asarray(b2, np.float32)
    V1 = np.asarray(V1, np.float32); c1 = np.asarray(c1, np.float32)
    V2 = np.asarray(V2, np.float32); c2 = np.asarray(c2, np.float32)

    in_maps = None
    for W in (8, 32):
        in_maps, ok = _prep_inputs(state, arch, energy, phi_local, surprise,
                                   seg_ids, W, W1, b1, W2, b2, V1, c1, V2, c2)
        if ok:
            break
    assert in_maps is not None, "no feasible scatter window schedule"

    nc = _get_program(W)
    outs = _run(nc, in_maps, W)
    couts = [np.asarray(outs[c]["out_cluster"]) for c in range(NCORES)]
    orgs = [np.asarray(outs[c]["out_org"]).reshape(12) for c in range(NCORES)]
    cluster_full = np.concatenate(couts, 0).astype(np.float32)
    p = np.sum(np.stack(orgs, 0), 0, dtype=np.float64)
    Z, G, sphi, scoh, nval, pres = p[0], p[1:5], p[5], p[6], p[7], p[8:12]
    ga = (G / Z).astype(np.float32)
    e = np.exp(ga - ga.max())
    global_arch = (e / e.sum()).astype(np.float32)
    n_valid = max(nval, 1.0)
    avg_phi = sphi / n_valid
    unique = float((pres > 0).sum())
    phi_global = min(1.0, avg_phi * (0.5 + 0.5 * unique / 4.0))
    vert = scoh / n_valid
    self_model = np.array([*global_arch, phi_global, vert], np.float32)
    return np.concatenate([cluster_full.reshape(-1), self_model]).astype(np.float32)


# revision 19
# speedup vs baseline: 1.0295x; 1.0295x over previous
"""Trainium2 Bass kernel for nn_BottomUpIntegrator (gnn_message_passing).

Sharding: cells split at cluster boundaries across 8 cores (2048 clusters
each); per-core segment sums via one-hot scatter matmuls into PSUM with a
core-invariant static window schedule; cluster+organism phase on-chip; host
combines 12 organism partial floats per core into the final 6 self-model
outputs.

v2: fp8 feats with b1 folded in via a ones row, single ACT table
(tanh-based sigmoid), no in-loop barriers, W=8 scatter windows,
superchunk DMA with 16-32KB descriptors, ACT/Pool/DVE engine split,
vectorized host prep, memoized jit executable.
"""
import os
import numpy as np
import ml_dtypes

import json as _json

from concourse import bass, mybir
from concourse import bass2jax as _b2j
from concourse import bass_utils as _bu
from concourse.tile import TileContext
from concourse.bass_utils import run_bass_kernel_spmd

_orig_compile = _bu.compile_bir_kernel


def _split_waits_compile(bir_json, tmpdir, neff_name="file.neff"):
    """Walrus lowers at most ONE semaphore wait per TPB instruction struct.
    Tile emits several. Hoist extras onto injected same-engine EventSemaphore
    wait instructions immediately before the owner (semantically identical:
    engines execute in program order)."""
    d = _json.loads(bir_json)
    cnt = 0
    for fn in d["functions"]:
        for blk in fn["blocks"]:
            newlist = []
            for ins in blk["instructions"]:
                si = ins.get("sync_info")
                waits = si.get("on_wait", []) if si else []
                if si and len(waits) > 1 and ins.get("opcode") not in (
                        "EventSemaphore",):
                    for w_i, w in enumerate(waits[:-1]):
                        cnt += 1
                        newlist.append({
                            "debug": ins.get("debug", 0),
                            "engine": ins["engine"],
                            "ins": [], "outs": [],
                            "name": f"{ins['name']}-wsplit{w_i}",
                            "opcode": "EventSemaphore",
                            "sync_info": {"on_update": [], "on_wait": [w]},
                        })
                    si["on_wait"] = [waits[-1]]
                newlist.append(ins)
            blk["instructions"] = newlist
    print(f"[wait-split] hoisted {cnt} extra waits")
    return _orig_compile(_json.dumps(d).encode(), tmpdir, neff_name=neff_name)


_bu.compile_bir_kernel = _split_waits_compile
_b2j.compile_bir_kernel = _split_waits_compile

F32 = mybir.dt.float32
BF16 = mybir.dt.bfloat16
FP8 = mybir.dt.float8e4
AF = mybir.ActivationFunctionType
OP = mybir.AluOpType
AX = mybir.AxisListType

NCORES = 8
KLOC = 2048            # clusters per core
NPAD = 262144          # padded cells per core
CHUNK = 8192           # cells per chunk
NCHUNK = NPAD // CHUNK # 32
SUPER = 4              # chunks per DMA superchunk
NSUP = NCHUNK // SUPER # 8
NTILES = NPAD // 128   # 2048 scatter tiles per core
TPB = NTILES // 4      # tiles per 512-cluster block
PADSEG = 1.0e9

FP8_NP = ml_dtypes.float8_e4m3
BF16_NP = ml_dtypes.bfloat16


def _window_starts(W):
    S = np.arange(NTILES)
    s = S % TPB
    return np.clip(s - W // 2, 0, TPB - W).astype(np.int64)


def build_program(W):
    nc = bass.Bass(trn_type="TRN2", use_seq_codegen=True)
    featsT = nc.dram_tensor("featsT", [40, NPAD], FP8, kind="ExternalInput")
    archcm = nc.dram_tensor("archcm", [128, NCHUNK * 576], BF16, kind="ExternalInput")
    cellvec = nc.dram_tensor("cellvec", [128, NCHUNK * 192], BF16, kind="ExternalInput")
    w1d = nc.dram_tensor("w1d", [40, 256], FP8, kind="ExternalInput")
    w2d = nc.dram_tensor("w2d", [128, 2], BF16, kind="ExternalInput")
    b2hd = nc.dram_tensor("b2hd", [128, 1], F32, kind="ExternalInput")
    iotat = nc.dram_tensor("iotat", [128, 64 * W], BF16, kind="ExternalInput")
    ident = nc.dram_tensor("ident", [128, 128], F32, kind="ExternalInput")
    v1 = nc.dram_tensor("v1", [7, 32], F32, kind="ExternalInput")
    c1b = nc.dram_tensor("c1b", [32, 1], F32, kind="ExternalInput")
    v2 = nc.dram_tensor("v2", [32, 1], F32, kind="ExternalInput")
    c2h = nc.dram_tensor("c2h", [1, 1], F32, kind="ExternalInput")
    out_cluster = nc.dram_tensor("out_cluster", [KLOC, 8], F32, kind="ExternalOutput")
    out_org = nc.dram_tensor("out_org", [1, 12], F32, kind="ExternalOutput")

    wstart = _window_starts(W)

    with TileContext(nc) as tc:
        with (
            tc.tile_pool(name="const", bufs=1) as cp,
            tc.tile_pool(name="feats", bufs=3) as fp,
            tc.tile_pool(name="acp", bufs=4) as ap_,
            tc.tile_pool(name="cvp", bufs=4) as vp_,
            tc.tile_pool(name="hs", bufs=2) as hp,
            tc.tile_pool(name="small", bufs=4) as sp,
            tc.tile_pool(name="scatv", bufs=2) as vp,
            tc.tile_pool(name="ohp", bufs=2) as op_,
            tc.tile_pool(name="ph_b", bufs=1) as bp,
            tc.tile_pool(name="scatps", bufs=1, space="PSUM") as pps,
        ):
            # ---- constants ----------------------------------------------
            w1s = cp.tile([40, 256], FP8, tag="w1s")
            nc.sync.dma_start(out=w1s[:], in_=w1d[:])
            w2s = cp.tile([128, 2], BF16, tag="w2s")
            nc.sync.dma_start(out=w2s[:], in_=w2d[:])
            b2hs = cp.tile([128, 1], F32, tag="b2hs")
            nc.sync.dma_start(out=b2hs[:], in_=b2hd[:])
            iots = cp.tile([128, 64 * W], BF16, tag="iots")
            nc.sync.dma_start(out=iots[:], in_=iotat[:])
            ids = cp.tile([128, 128], F32, tag="ids")
            nc.sync.dma_start(out=ids[:], in_=ident[:])
            v1s = cp.tile([7, 32], F32, tag="v1s")
            nc.sync.dma_start(out=v1s[:], in_=v1[:])
            c1s = cp.tile([32, 1], F32, tag="c1s")
            nc.sync.dma_start(out=c1s[:], in_=c1b[:])
            v2s = cp.tile([32, 1], F32, tag="v2s")
            nc.sync.dma_start(out=v2s[:], in_=v2[:])
            c2hs = cp.tile([1, 1], F32, tag="c2hs")
            nc.sync.dma_start(out=c2hs[:], in_=c2h[:])
            ones = cp.tile([128, 1], F32, tag="ones")
            nc.vector.memset(ones[:], 1.0)

            zbf = cp.tile([128, 512], BF16, tag="zbf")
            nc.vector.memset(zbf[:], 0.0)

            # Pre-touch DMA-loaded constants on their consuming engines.
            scra = cp.tile([128, 4], F32, tag="scra")
            nc.scalar.activation(out=scra[:, 0:1], in_=b2hs[:], func=AF.Copy)
            nc.scalar.activation(out=scra[0:32, 1:2], in_=c1s[:], func=AF.Copy)
            nc.scalar.activation(out=scra[0:1, 2:3], in_=c2hs[:], func=AF.Copy)
            scrv = cp.tile([1, 2], F32, tag="scrv")
            nc.vector.tensor_copy(out=scrv[:, 0:1], in_=iots[0:1, 0:1])
            nc.gpsimd.tensor_copy(out=scrv[:, 1:2], in_=iots[0:1, 1:2])

            # persistent scatter accumulators: 4 PSUM banks of [15, 512]
            scat = [pps.tile([15, 512], F32, tag=f"scat{b}", name=f"scat{b}")
                    for b in range(4)]
            nc.tensor.matmul(out=scat[0][0:1, 0:1], lhsT=ids[0:1, 0:1],
                             rhs=ids[0:1, 0:1], start=True, stop=True,
                             skip_group_check=True)
            for b in range(4):
                nc.tensor.matmul(out=scat[b][:], lhsT=zbf[:, 0:15], rhs=zbf[:],
                                 start=True, stop=False, skip_group_check=True)

            # ---- phase A ------------------------------------------------
            with (
                tc.tile_pool(name="mm1ps", bufs=2, space="PSUM") as pp1,
                tc.tile_pool(name="mm2ps", bufs=2, space="PSUM") as pp2,
            ):
                for s in range(NSUP):
                    ft = fp.tile([40, SUPER * 8192], FP8, tag="ft")
                    c0 = s * SUPER * 8192
                    if s == 0:
                        # chunk-granular first loads so compute starts sooner
                        for i4 in range(SUPER):
                            nc.sync.dma_start(
                                out=ft[:, i4 * 8192:(i4 + 1) * 8192],
                                in_=featsT[:, c0 + i4 * 8192:c0 + (i4 + 1) * 8192])
                    else:
                        nc.sync.dma_start(
                            out=ft[:], in_=featsT[:, c0:c0 + SUPER * 8192],
                            max_dma_last_dim=16384)
                    ac = ap_.tile([128, SUPER * 576], BF16, tag="ac")
                    nc.sync.dma_start(
                        out=ac[:], in_=archcm[:, s * SUPER * 576:(s + 1) * SUPER * 576])
                    cv = vp_.tile([128, SUPER * 192], BF16, tag="cv")
                    nc.sync.dma_start(
                        out=cv[:], in_=cellvec[:, s * SUPER * 192:(s + 1) * SUPER * 192])

                    for i in range(SUPER):
                        k = s * SUPER + i
                        acs = ac[:, i * 576:(i + 1) * 576]
                        cvs = cv[:, i * 192:(i + 1) * 192]
                        ftc = ft[:, i * 8192:(i + 1) * 8192].rearrange(
                            "p (k2 q) -> p k2 q", k2=2)
                        w1s3 = w1s[:].rearrange("p (k2 m) -> p k2 m", k2=2)

                        # mm1 (fp8 DoubleRow) + relu -> hs [128, 4096] bf16
                        hs = hp.tile([128, 4096], BF16, tag="hs")
                        for j in range(8):
                            hp1 = pp1.tile([128, 512], F32, tag="hp1")
                            nc.tensor.matmul(out=hp1[:], lhsT=w1s3,
                                             rhs=ftc[:, :, 512 * j:512 * (j + 1)],
                                             start=True, stop=True,
                                             perf_mode=mybir.MatmulPerfMode.DoubleRow)
                            dst = hs[:, 512 * j:512 * (j + 1)]
                            if j % 2 == 0:
                                nc.scalar.activation(out=dst, in_=hp1[:], func=AF.Relu)
                            else:
                                nc.vector.tensor_scalar(out=dst, in0=hp1[:],
                                                        scalar1=0.0, scalar2=None,
                                                        op0=OP.max)

                        # mm2 -> base preact [128, 64] cell-major in PSUM
                        bb = pp2.tile([128, 64], F32, tag="bb")
                        for t in range(32):
                            nc.tensor.matmul(out=bb[:, 2 * t:2 * t + 2],
                                             lhsT=hs[:, 128 * t:128 * (t + 1)], rhs=w2s[:],
                                             start=(t == 0), stop=(t == 31),
                                             skip_group_check=True)

                        # sigmoid via tanh: sig = 0.5*tanh(0.5 x + 0.5 b2) + 0.5
                        sg = sp.tile([128, 64], F32, tag="sg")
                        nc.scalar.activation(out=sg[:], in_=bb[:], func=AF.Tanh,
                                             bias=b2hs[:], scale=0.5)
                        # imp = clip((t+1) * eph2, .01, 1);  w = imp * eph
                        imp = sp.tile([128, 64], F32, tag="imp")
                        nc.vector.scalar_tensor_tensor(
                            out=imp[:], in0=sg[:], scalar=1.0,
                            in1=cvs[:, 64:128], op0=OP.add, op1=OP.mult)
                        nc.vector.tensor_scalar(out=imp[:], in0=imp[:], scalar1=0.01,
                                                scalar2=1.0, op0=OP.max, op1=OP.min)
                        wq = sp.tile([128, 64], F32, tag="wq")
                        nc.vector.tensor_tensor(out=wq[:], in0=imp[:], in1=cvs[:, 0:64],
                                                op=OP.mult)

                        # values [128, 64*15] bf16
                        # value-major vt: [128, 15v, 64s] -> contiguous writes
                        vt = vp.tile([128, 15 * 64], BF16, tag="vt")
                        vv = vt[:].rearrange("p (v s) -> p v s", s=64)
                        nc.gpsimd.memset(vv[:, 0:1, :], 1.0)
                        # exp writes e^w straight into vt value-slot 1 (bf16)
                        nc.scalar.activation(out=vt[:, 64:128], in_=wq[:],
                                             func=AF.Exp)
                        nc.vector.tensor_tensor(
                            out=vv[:, 2:6, :],
                            in0=acs[:, 0:256].rearrange("p (a s) -> p a s", s=64),
                            in1=vt[:, 64:128].to_broadcast([128, 64, 4]).rearrange(
                                "p s a -> p a s"),
                            op=OP.mult)
                        # a, a^2, su land as one contiguous block copy
                        nc.scalar.activation(out=vt[:, 6 * 64:15 * 64],
                                             in_=acs[:, 0:576], func=AF.Copy)

                        # one-hot [128, 64*W] bf16 (DVE)
                        oh = op_.tile([128, 64 * W], BF16, tag="oh")
                        ohv = oh[:].rearrange("p (s w) -> p s w", w=W)
                        iov = iots[:].rearrange("p (s w) -> p s w", w=W)
                        nc.vector.tensor_tensor(
                            out=ohv[:, :, :], in0=iov[:, :, :],
                            in1=cvs[:, 128:192].to_broadcast([128, 64, W]),
                            op=OP.is_equal)

                        # scatter: col j -> sorted tile S = 64k + 32*(j%2) + j//2
                        # lhsT strided: value-major vt, tile j's 15 values at
                        # stride 64
                        for j in range(64):
                            S = 64 * k + 32 * (j % 2) + (j // 2)
                            blk = S // TPB
                            f = int(wstart[S])
                            nc.tensor.matmul(out=scat[blk][:, f:f + W],
                                             lhsT=vv[:, :, j:j + 1],
                                             rhs=oh[:, W * j:W * j + W],
                                             start=False,
                                             stop=(j == 63 and k % 8 == 7),
                                             skip_group_check=True)

            # ---- phase B ------------------------------------------------
            tc.strict_bb_all_engine_barrier()
            sc = bp.tile([15, 2048], F32, tag="sc")
            for b in range(4):
                nc.vector.tensor_copy(out=sc[:, 512 * b:512 * (b + 1)], in_=scat[b][:])

            with (
                tc.tile_pool(name="ptps", bufs=2, space="PSUM") as ppt,
                tc.tile_pool(name="mmbps", bufs=2, space="PSUM") as ppm,
            ):
                tt = bp.tile([128, 16 * 15], F32, tag="tt")
                for b in range(16):
                    pt = ppt.tile([128, 15], F32, tag="pt")
                    nc.tensor.transpose(out=pt[:], in_=sc[:, 128 * b:128 * (b + 1)],
                                        identity=ids[0:15, 0:15])
                    nc.vector.tensor_copy(out=tt[:, 15 * b:15 * (b + 1)], in_=pt[:])
                tv = tt[:].rearrange("p (b q) -> p b q", q=15)
                cnt = tv[:, :, 0:1]      # [128,16,1]
                sew = tv[:, :, 1:2]
                sewa = tv[:, :, 2:6]
                sa = tv[:, :, 6:10]
                ssq = tv[:, :, 10:14]
                ssur = tv[:, :, 14:15]

                def wt(tag):
                    return bp.tile([128, 16], F32, tag=tag, name=tag)

                def v3(t):
                    return t[:].rearrange("p (b a) -> p b a", a=1)

                def w4(tag):
                    t = bp.tile([128, 64], F32, tag=tag, name=tag)
                    return t, t[:].rearrange("p (b a) -> p b a", a=4)

                cntc = wt("cntc")
                nc.vector.tensor_scalar(out=v3(cntc), in0=cnt, scalar1=1.0,
                                        scalar2=None, op0=OP.max)
                rc = wt("rc")
                nc.vector.reciprocal(out=rc[:], in_=cntc[:])
                den = wt("den")
                nc.vector.tensor_scalar(out=v3(den), in0=sew, scalar1=1.0,
                                        scalar2=None, op0=OP.max)
                rden = wt("rden")
                nc.vector.reciprocal(out=rden[:], in_=den[:])
                agr, agrv = w4("agr")
                nc.vector.tensor_tensor(out=agrv, in0=sewa,
                                        in1=rden[:].to_broadcast([128, 16, 4]),
                                        op=OP.mult)
                mx = wt("mx")
                nc.vector.tensor_reduce(out=v3(mx), in_=agrv, axis=AX.X, op=OP.max)
                es, esv = w4("es")
                nc.vector.tensor_tensor(out=esv, in0=agrv,
                                        in1=mx[:].to_broadcast([128, 16, 4]),
                                        op=OP.subtract)
                nc.scalar.activation(out=es[:], in_=es[:], func=AF.Exp)
                ssum = wt("ssum")
                nc.vector.tensor_reduce(out=v3(ssum), in_=esv, axis=AX.X, op=OP.add)
                rssum = wt("rssum")
                nc.vector.reciprocal(out=rssum[:], in_=ssum[:])
                agg, aggv = w4("agg")
                nc.vector.tensor_tensor(out=aggv, in0=esv,
                                        in1=rssum[:].to_broadcast([128, 16, 4]),
                                        op=OP.mult)
                mean, meanv = w4("mean")
                nc.vector.tensor_tensor(out=meanv, in0=sa,
                                        in1=rc[:].to_broadcast([128, 16, 4]),
                                        op=OP.mult)
                var, varv = w4("var")
                nc.vector.tensor_tensor(out=varv, in0=meanv, in1=meanv, op=OP.mult)
                cntb = wt("cntb")
                nc.vector.tensor_copy(out=v3(cntb), in_=cnt)
                nc.vector.tensor_tensor(out=varv, in0=varv,
                                        in1=cntb[:].to_broadcast([128, 16, 4]),
                                        op=OP.mult)
                nc.vector.tensor_tensor(out=varv, in0=ssq, in1=varv, op=OP.subtract)
                cm1 = wt("cm1")
                nc.vector.tensor_scalar(out=v3(cm1), in0=cnt, scalar1=-1.0,
                                        scalar2=1.0, op0=OP.add, op1=OP.max)
                rcm1 = wt("rcm1")
                nc.vector.reciprocal(out=rcm1[:], in_=cm1[:])
                nc.vector.tensor_tensor(out=varv, in0=varv,
                                        in1=rcm1[:].to_broadcast([128, 16, 4]),
                                        op=OP.mult)
                vm = wt("vm")
                nc.vector.tensor_reduce(out=v3(vm), in_=varv, axis=AX.X, op=OP.add)
                nc.vector.tensor_scalar(out=vm[:], in0=vm[:], scalar1=0.25,
                                        scalar2=None, op0=OP.mult)
                phic = wt("phic")
                nc.vector.tensor_scalar(out=phic[:], in0=vm[:], scalar1=2.0,
                                        scalar2=1.0, op0=OP.mult, op1=OP.min)
                nc.vector.tensor_scalar(out=phic[:], in0=phic[:], scalar1=-1.0,
                                        scalar2=1.0, op0=OP.mult, op1=OP.add)
                coh = wt("coh")
                nc.vector.tensor_scalar(out=coh[:], in0=vm[:], scalar1=-1.0,
                                        scalar2=1.0, op0=OP.mult, op1=OP.add)
                perr = wt("perr")
                nc.vector.tensor_tensor(out=v3(perr), in0=ssur, in1=v3(rc),
                                        op=OP.mult)
                integ = wt("integ")
                nc.vector.tensor_scalar(out=integ[:], in0=perr[:], scalar1=-1.0,
                                        scalar2=1.0, op0=OP.mult, op1=OP.add)
                nc.vector.tensor_tensor(out=integ[:], in0=integ[:], in1=phic[:],
                                        op=OP.mult)

                # cluster MLP
                cft = bp.tile([128, 16 * 7], F32, tag="cft")
                cfv = cft[:].rearrange("p (b q) -> p b q", q=7)
                nc.vector.tensor_copy(out=cfv[:, :, 0:4], in_=aggv)
                nc.vector.tensor_copy(out=cfv[:, :, 4:5],
                                      in_=phic[:].to_broadcast([128, 16, 1]))
                nc.vector.tensor_copy(out=cfv[:, :, 5:6],
                                      in_=coh[:].to_broadcast([128, 16, 1]))
                szf = wt("szf")
                nc.vector.tensor_scalar(out=v3(szf), in0=cnt, scalar1=0.05,
                                        scalar2=1.0, op0=OP.mult, op1=OP.min)
                nc.vector.tensor_copy(out=cfv[:, :, 6:7],
                                      in_=szf[:].to_broadcast([128, 16, 1]))
                cftt = bp.tile([7, 2048], F32, tag="cftt")
                for b in range(16):
                    ptc = ppt.tile([128, 128], F32, tag="pt")
                    nc.tensor.transpose(out=ptc[0:7, :],
                                        in_=cft[:, 7 * b:7 * (b + 1)], identity=ids[:])
                    nc.vector.tensor_copy(out=cftt[:, 128 * b:128 * (b + 1)],
                                          in_=ptc[0:7, :])
                hcs = bp.tile([32, 2048], F32, tag="hcs")
                for i in range(4):
                    hcp = ppm.tile([32, 512], F32, tag="mmp")
                    nc.tensor.matmul(out=hcp[:], lhsT=v1s[:],
                                     rhs=cftt[:, 512 * i:512 * (i + 1)],
                                     start=True, stop=True)
                    nc.scalar.activation(out=hcs[:, 512 * i:512 * (i + 1)],
                                         in_=hcp[:], func=AF.Relu, bias=c1s[:])
                # basec tanh preact
                sgc = bp.tile([1, 2048], F32, tag="sgc")
                for i in range(4):
                    bcp = ppm.tile([32, 512], F32, tag="mmp")
                    nc.tensor.matmul(out=bcp[0:1, :], lhsT=v2s[:],
                                     rhs=hcs[:, 512 * i:512 * (i + 1)],
                                     start=True, stop=True)
                    nc.scalar.activation(out=sgc[:, 512 * i:512 * (i + 1)],
                                         in_=bcp[0:1, :], func=AF.Tanh,
                                         bias=c2hs[:], scale=0.5)
                tct = wt("tct")
                for b in range(16):
                    ptb = ppt.tile([128, 128], F32, tag="pt")
                    nc.tensor.transpose(out=ptb[:, 0:1],
                                        in_=sgc[:, 128 * b:128 * (b + 1)],
                                        identity=ids[0:1, 0:1])
                    nc.vector.tensor_copy(out=tct[:, b:b + 1], in_=ptb[:, 0:1])
                # impc = clip(0.5*(t+1)*phic, .01, 1)
                impc = wt("impc")
                nc.vector.scalar_tensor_tensor(out=impc[:], in0=tct[:], scalar=1.0,
                                               in1=phic[:], op0=OP.add, op1=OP.mult)
                nc.vector.tensor_scalar(out=impc[:], in0=impc[:], scalar1=0.5,
                                        scalar2=0.01, op0=OP.mult, op1=OP.max)
                nc.vector.tensor_scalar(out=impc[:], in0=impc[:], scalar1=1.0,
                                        scalar2=None, op0=OP.min)
                valid = wt("valid")
                nc.vector.tensor_scalar(out=v3(valid), in0=cnt, scalar1=0.0,
                                        scalar2=None, op0=OP.is_gt)
                eimp = wt("eimp")
                nc.scalar.activation(out=eimp[:], in_=impc[:], func=AF.Exp)
                nc.vector.tensor_tensor(out=eimp[:], in0=eimp[:], in1=valid[:],
                                        op=OP.mult)
                amx = wt("amx")
                nc.vector.tensor_reduce(out=v3(amx), in_=aggv, axis=AX.X, op=OP.max)
                bsel, bselv = w4("bsel")
                nc.vector.tensor_tensor(out=bselv, in0=aggv,
                                        in1=amx[:].to_broadcast([128, 16, 4]),
                                        op=OP.is_equal)
                taken = wt("taken")
                nc.vector.memset(taken[:], 0.0)
                notk = wt("notk")
                for a in range(4):
                    nc.vector.tensor_scalar(out=notk[:], in0=taken[:], scalar1=-1.0,
                                            scalar2=1.0, op0=OP.mult, op1=OP.add)
                    nc.vector.tensor_tensor(out=bselv[:, :, a:a + 1],
                                            in0=bselv[:, :, a:a + 1], in1=v3(notk),
                                            op=OP.mult)
                    if a < 3:
                        nc.vector.tensor_tensor(out=v3(taken), in0=v3(taken),
                                                in1=bselv[:, :, a:a + 1], op=OP.max)
                # reductions -> R [128, 12]
                r = bp.tile([128, 12], F32, tag="r")
                ga, gav = w4("ga")
                nc.vector.tensor_tensor(out=gav, in0=aggv,
                                        in1=eimp[:].to_broadcast([128, 16, 4]),
                                        op=OP.mult)
                pv = wt("pv")
                nc.vector.tensor_tensor(out=pv[:], in0=phic[:], in1=valid[:],
                                        op=OP.mult)
                cvv = wt("cvv")
                nc.vector.tensor_tensor(out=cvv[:], in0=coh[:], in1=valid[:],
                                        op=OP.mult)
                bv, bvv = w4("bv")
                nc.vector.tensor_tensor(out=bvv, in0=bselv,
                                        in1=valid[:].to_broadcast([128, 16, 4]),
                                        op=OP.mult)
                nc.vector.tensor_reduce(out=r[:, 0:1], in_=eimp[:], axis=AX.X,
                                        op=OP.add)
                gat = ga[:].rearrange("p (b a) -> p a b", a=4)
                nc.vector.tensor_reduce(
                    out=r[:, 1:5].rearrange("p (a o) -> p a o", o=1),
                    in_=gat, axis=AX.X, op=OP.add)
                nc.vector.tensor_reduce(out=r[:, 5:6], in_=pv[:], axis=AX.X, op=OP.add)
                nc.vector.tensor_reduce(out=r[:, 6:7], in_=cvv[:], axis=AX.X,
                                        op=OP.add)
                nc.vector.tensor_reduce(out=r[:, 7:8], in_=valid[:], axis=AX.X,
                                        op=OP.add)
                bvt = bv[:].rearrange("p (b a) -> p a b", a=4)
                nc.vector.tensor_reduce(
                    out=r[:, 8:12].rearrange("p (a o) -> p a o", o=1),
                    in_=bvt, axis=AX.X, op=OP.add)
                orgp = ppm.tile([32, 512], F32, tag="mmp")
                nc.tensor.matmul(out=orgp[0:1, 0:12], lhsT=ones[:], rhs=r[:],
                                 start=True, stop=True)
                orgs = bp.tile([1, 12], F32, tag="orgs")
                nc.vector.tensor_copy(out=orgs[:], in_=orgp[0:1, 0:12])
                nc.sync.dma_start(out=out_org[:], in_=orgs[:])

                # cluster_out [2048, 8]
                oc = bp.tile([128, 128], F32, tag="oc")
                ocv = oc[:].rearrange("p (b q) -> p b q", q=8)
                nc.vector.tensor_copy(out=ocv[:, :, 0:4], in_=aggv)
                nc.vector.tensor_copy(out=ocv[:, :, 4:5],
                                      in_=phic[:].to_broadcast([128, 16, 1]))
                nc.vector.tensor_copy(out=ocv[:, :, 5:6],
                                      in_=coh[:].to_broadcast([128, 16, 1]))
                nc.vector.tensor_copy(out=ocv[:, :, 6:7],
                                      in_=perr[:].to_broadcast([128, 16, 1]))
                nc.vector.tensor_copy(out=ocv[:, :, 7:8],
                                      in_=integ[:].to_broadcast([128, 16, 1]))
                nc.sync.dma_start(
                    out=out_cluster[:].rearrange("(b p) q -> p b q", p=128), in_=ocv)
    return nc


_NC_CACHE = {}


def _get_program(W):
    if W not in _NC_CACHE:
        _NC_CACHE[W] = build_program(W)
    return _NC_CACHE[W]


# --------------------------------------------------------------------------
# host-side prep (vectorized)
# --------------------------------------------------------------------------

def _schedule_core(lseg, W):
    """Static window schedule for one core. Returns (dst, rel, ok)."""
    Nc = lseg.shape[0]
    S = np.arange(NTILES)
    f = _window_starts(W)
    wlo = 512 * (S // TPB) + f
    whi = wlo + W
    a = np.searchsorted(lseg, whi)
    b = a - 128 * S
    runmin = np.minimum.accumulate(b)
    cur_next = np.minimum(runmin + 128 * S, 128 * (S + 1))
    cur = np.concatenate([[0], cur_next[:-1]])
    take = (cur_next - cur).astype(np.int64)
    if cur_next[-1] != Nc:
        return None, None, False
    m = take > 0
    first_idx = np.minimum(cur, Nc - 1)
    if np.any((lseg[first_idx] < wlo) & m):
        return None, None, False
    tile_of = np.repeat(S, take)
    dst = np.repeat(128 * S - cur, take) + np.arange(Nc)
    rel = (lseg - wlo[tile_of]).astype(np.float32)
    return dst, rel, True


def _prep_inputs(state, arch, energy, phi_local, surprise, seg_ids, W,
                 W1, b1, W2, b2, V1, c1, V2, c2):
    N = state.shape[0]
    fp8_one = np.float32(1.0).astype(FP8_NP).view(np.uint8)

    # global conversions (once)
    f8s = state.astype(FP8_NP).view(np.uint8)          # [N, 32]
    f8a = arch.astype(FP8_NP).view(np.uint8)           # [N, 4]
    arch_bf = arch.astype(BF16_NP).view(np.uint16)     # [N, 4]
    archsq_bf = (arch * arch).astype(BF16_NP).view(np.uint16)
    eph = energy * phi_local
    eph_bf = eph.astype(BF16_NP).view(np.uint16)
    eph2_bf = (0.5 * eph).astype(BF16_NP).view(np.uint16)
    su_bf = surprise.astype(BF16_NP).view(np.uint16)

    w1d80 = np.zeros((80, 128), np.float32)
    w1d80[0:36, 0:64] = W1
    w1d80[36:72, 64:128] = W1
    w1d80[72, 0:64] = b1
    w1d80[72, 64:128] = b1
    # DoubleRow packing: [40, 2, 128] with k2 = row // 40
    w1d = w1d80.reshape(2, 40, 128).transpose(1, 0, 2).reshape(40, 256)
    w2d = np.zeros((128, 2), np.float32)
    w2d[0:64, 0] = W2[:, 0]
    w2d[64:128, 1] = W2[:, 0]
    iota = np.tile(np.arange(W, dtype=np.float32), 64)
    consts = dict(
        w1d=w1d.astype(FP8_NP),
        w2d=w2d.astype(BF16_NP),
        b2hd=np.full((128, 1), 0.5 * b2[0], np.float32),
        iotat=np.ascontiguousarray(
            np.broadcast_to(iota.astype(BF16_NP), (128, 64 * W))),
        ident=np.eye(128, dtype=np.float32),
        v1=V1, c1b=c1.reshape(32, 1), v2=V2,
        c2h=np.full((1, 1), 0.5 * c2[0], np.float32),
    )

    bounds = np.searchsorted(seg_ids, np.arange(0, 16384 + 1, KLOC))
    in_maps = []
    for c in range(NCORES):
        B0, B1 = int(bounds[c]), int(bounds[c + 1])
        lseg = (seg_ids[B0:B1] - KLOC * c).astype(np.int64)
        dst, rel, ok = _schedule_core(lseg, W)
        if not ok:
            return None, None
        Nc = B1 - B0

        # padded feats [NPAD, 36] fp8-bytes -> featsT [73, NPAD//2]
        pad36 = np.zeros((NPAD, 36), np.uint8)
        pad36[dst, :32] = f8s[B0:B1]
        pad36[dst, 32:] = f8a[B0:B1]
        fT = np.zeros((80, NPAD // 2), np.uint8)
        srcp = pad36.reshape(NCHUNK, 2, 4096, 36).transpose(1, 3, 0, 2)
        fTv = fT[:72].reshape(2, 36, NCHUNK, 4096)
        np.copyto(fTv, srcp)
        fT[72] = fp8_one
        # DoubleRow: [40, NCHUNK, k2, 4096]
        fT = np.ascontiguousarray(
            fT.reshape(2, 40, NCHUNK, 4096).transpose(1, 2, 0, 3)).reshape(
            40, NPAD)

        # archcm [128, NCHUNK*576] bf16: per chunk value-major [a | a^2 | su]
        acm_u = np.empty((128, NCHUNK, 9, 64), np.uint16)
        apad = np.zeros((NPAD, 4), np.uint16)
        apad[dst] = arch_bf[B0:B1]
        acm_u[:, :, 0:4] = (apad.reshape(NCHUNK, 2, 32, 128, 4)
                            .transpose(3, 0, 4, 2, 1).reshape(128, NCHUNK, 4, 64))
        apad[:] = 0
        apad[dst] = archsq_bf[B0:B1]
        acm_u[:, :, 4:8] = (apad.reshape(NCHUNK, 2, 32, 128, 4)
                            .transpose(3, 0, 4, 2, 1).reshape(128, NCHUNK, 4, 64))
        spad = np.zeros(NPAD, np.uint16)
        spad[dst] = su_bf[B0:B1]
        acm_u[:, :, 8] = spad.reshape(NCHUNK, 2, 32, 128).transpose(
            3, 0, 2, 1).reshape(128, NCHUNK, 64)
        acm = acm_u.reshape(128, NCHUNK * 576).view(BF16_NP)

        # cellvec [128, NCHUNK*256] bf16: [eph | eph2 | su | rel] per chunk
        def swz(x_pad):
            return np.ascontiguousarray(
                x_pad.reshape(NCHUNK, 2, 32, 128).transpose(3, 0, 2, 1)
            ).reshape(128, NCHUNK, 64)

        cvb = np.empty((128, NCHUNK, 3, 64), np.uint16)
        epad = np.zeros(NPAD, np.uint16)
        epad[dst] = eph_bf[B0:B1]
        cvb[:, :, 0, :] = swz(epad)
        epad2 = np.zeros(NPAD, np.uint16)
        epad2[dst] = eph2_bf[B0:B1]
        cvb[:, :, 1, :] = swz(epad2)
        rpad = np.full(NPAD, PADSEG, np.float32)
        rpad[dst] = rel
        cvb[:, :, 2, :] = swz(rpad.astype(BF16_NP).view(np.uint16))
        cvv = cvb.reshape(128, NCHUNK * 192).view(BF16_NP)

        in_maps.append(dict(featsT=fT.view(FP8_NP),
                            archcm=acm,
                            cellvec=np.ascontiguousarray(cvv), **consts))
    return in_maps, True


# --------------------------------------------------------------------------
# cached PJRT execution (avoids re-trace/re-compile on repeat calls)
# --------------------------------------------------------------------------

_EXEC_CACHE = {}


def _build_exec(nc):
    import jax
    from concourse.bass2jax import (_bass_exec_p, install_neuronx_cc_hook,
                                    partition_id_tensor)
    from jax.sharding import Mesh, PartitionSpec
    try:
        from jax.experimental.shard_map import shard_map
    except ImportError:
        from jax import shard_map

    install_neuronx_cc_hook()
    partition_name = nc.partition_id_tensor.name if nc.partition_id_tensor else None
    in_names = []
    out_names = []
    out_avals = []
    zero_shapes = []
    for alloc in nc.m.functions[0].allocations:
        if not isinstance(alloc, mybir.MemoryLocationSet):
            continue
        name = alloc.memorylocations[0].name
        if alloc.kind == "ExternalInput":
            if name != partition_name:
                in_names.append(name)
        elif alloc.kind == "ExternalOutput":
            out_names.append(name)
            shape = tuple(alloc.tensor_shape)
            dtype = mybir.dt.np(alloc.dtype)
            out_avals.append(jax.core.ShapedArray(shape, dtype))
            zero_shapes.append((shape, dtype))
    n_params = len(in_names)
    n_outs = len(out_names)
    all_names = in_names + out_names
    if partition_name is not None:
        all_names = all_names + [partition_name]

    def _body(*args):
        operands = list(args)
        if partition_name is not None:
            operands.append(partition_id_tensor())
        outs = _bass_exec_p.bind(
            *operands,
            out_avals=tuple(out_avals),
            in_names=tuple(all_names),
            out_names=tuple(out_names),
            lowering_input_output_aliases=(),
            sim_require_finite=True,
            sim_require_nnan=True,
            nc=nc,
        )
        return tuple(outs)

    devices = jax.devices()[:NCORES]
    mesh = Mesh(np.asarray(devices), ("core",))
    in_specs = (PartitionSpec("core"),) * (n_params + n_outs)
    out_specs = (PartitionSpec("core"),) * n_outs
    donate = tuple(range(n_params, n_params + n_outs))
    sharded = jax.jit(
        shard_map(_body, mesh=mesh, in_specs=in_specs, out_specs=out_specs,
                  check_rep=False),
        donate_argnums=donate, keep_unused=True)

    def run(in_maps):
        concat_in = [
            np.concatenate([np.asarray(in_maps[c][name]) for c in range(NCORES)],
                           axis=0)
            for name in in_names
        ]
        concat_zeros = [
            np.zeros((NCORES * s[0], *s[1:]), d) for s, d in zero_shapes
        ]
        out_arrs = sharded(*concat_in, *concat_zeros)
        return [
            {name: np.asarray(out_arrs[i]).reshape(NCORES, *out_avals[i].shape)[c]
             for i, name in enumerate(out_names)}
            for c in range(NCORES)
        ]

    return run


def _run(nc, in_maps, W):
    if os.environ.get("BASS_TRACE_RUN"):
        res = run_bass_kernel_spmd(nc, in_maps, list(range(NCORES)))
        return res.results
    if W not in _EXEC_CACHE:
        _EXEC_CACHE[W] = _build_exec(nc)
    return _EXEC_CACHE[W](in_maps)


def kernel(state, arch, energy, phi_local, surprise, seg_ids, n_clusters,
           W1, b1, W2, b2, V1, c1, V2, c2):
    state = np.asarray(state, np.float32)
    arch = np.asarray(arch, np.float32)
    energy = np.asarray(energy, np.float32)
    phi_local = np.asarray(phi_local, np.float32)
    surprise = np.asarray(surprise, np.float32)
    seg_ids = np.asarray(seg_ids)
    W1 = np.asarray(W1, np.float32); b1 = np.asarray(b1, np.float32)
    W2 = np.asarray(W2, np.float32); b2 = np.asarray(b2, np.float32)
    V1 = np.asarray(V1, np.float32); c1 = np.asarray(c1, np.float32)
    V2 = np.asarray(V2, np.float32); c2 = np.asarray(c2, np.float32)

    in_maps = None
    for W in (8, 32):
        in_maps, ok = _prep_inputs(state, arch, energy, phi_local, surprise,
                                   seg_ids, W, W1, b1, W2, b2, V1, c1, V2, c2)
        if ok:
            break
    assert in_maps is not None, "no feasible scatter window schedule"

    nc = _get_program(W)
    outs = _run(nc, in_maps, W)
    couts = [np.asarray(outs[c]["out_cluster"]) for c in range(NCORES)]
    orgs = [np.asarray(outs[c]["out_org"]).reshape(12) for c in range(NCORES)]
    cluster_full = np.concatenate(couts, 0).astype(np.float32)
    p = np.sum(np.stack(orgs, 0), 0, dtype=np.float64)
    Z, G, sphi, scoh, nval, pres = p[0], p[1:5], p[5], p[6], p[7], p[8:12]
    ga = (G / Z).astype(np.float32)
    e = np.exp(ga - ga.max())
    global_arch = (e / e.sum()).astype(np.float32)
    n_valid = max(nval, 1.0)
    avg_phi = sphi / n_valid
    unique = float((pres > 0).sum())
    phi_global = min(1.0, avg_phi * (0.5 + 0.5 * unique / 4.0))
    vert = scoh / n_valid
    self_model = np.array([*global_arch, phi_global, vert], np.float32)
    return np.concatenate([cluster_full.reshape(-1), self_model]).astype(np.float32)
